# revision 7
# baseline (speedup 1.0000x reference)
"""Trainium2 Bass kernel for DifferentialMultiHeadSelfAttention.

Sharding: 16 heads -> 8 cores (2 heads/core, tensor parallel).

v2 design:
- Scores per (head, 512-col chunk, t-tile): stationary kT [64,128], moving qT.
- exp on ScalarE into fp16 tiles [t, s]; causal trim at 128 granularity;
  diag/hole blocks masked by DVE multiply with keep tiles.
- AV flipped: stationary v_aug [t, 64+1] (ones col -> row sums), moving exp
  -> oT [65, s-chunk] accumulated in PSUM. Output lands directly in
  [channel, s] layout: no GroupNorm transposes at all.
- Softmax normalization: 1/rowsum broadcast across partitions with a C=1
  matmul (stationary ones / -lambda), then DVE combine of the two matrices.
- GroupNorm affine is deferred to the receiving core: the AllToAll payload
  carries 2 extra fp16 columns per block (inv_sigma, -mu). Chunk slices
  stream into a2a staging during attention; the tail is collective + affine
  + short output linear.

Shapes (hardcoded): B=1, S=2048, E=1024, H=16, DH=64.
"""
import numpy as np

from concourse import bacc, mybir, tile
from concourse.bass_utils import run_bass_kernel_spmd

# Pin all ScalarE activations to the one table set that covers every function
# used here (Exp, Ln) so the table never reloads.
_orig_gat = bacc.get_activation_tables


def _single_set_tables(arch):
    t = _orig_gat(arch)
    target = t.get("natural_log_exp_and_others")
    if target is None:
        return t
    out = {}
    for name, fns in t.items():
        if name == "natural_log_exp_and_others":
            out[name] = fns
        else:
            kept = {f for f in fns if f not in target}
            out[name] = kept if kept else set(fns)
    return out


bacc.get_activation_tables = _single_set_tables

dt = mybir.dt

NCORES = 8
S = 2048
E = 1024
H = 16
DH = 64
HLOC = H // NCORES          # heads per core = 2
SLOC = S // NCORES          # output rows per core = 256
NT = S // 128               # 16 t-tiles
NCH = S // 512              # 4 s-chunks
EPS = 1e-5

_PROGRAM_CACHE = {}


def _classify_mask(mask):
    """mask[s, t] bool, True = masked. Returns per-(t_tile, s_tile) state:
    0 = fully masked (skip), 1 = fully unmasked, 2 = needs mask multiply;
    plus the first-active s-tile per t-tile and keep (0/1) tiles in [t, s]
    layout for the state-2 blocks."""
    m = mask.reshape(NT, 128, NT, 128)  # [s_tile, s_in, t_tile, t_in]
    state = np.empty((NT, NT), dtype=np.int32)  # [t_tile, s_tile]
    keep_tiles = []
    tile_idx = -np.ones((NT, NT), dtype=np.int32)
    for t in range(NT):
        for s in range(NT):
            blk = m[s, :, t, :]  # [s_in, t_in]
            if blk.all():
                state[t, s] = 0
            elif not blk.any():
                state[t, s] = 1
            else:
                state[t, s] = 2
                tile_idx[t, s] = len(keep_tiles)
                keep_tiles.append((~blk.T).astype(np.float16))  # [t_in, s_in]
    s0 = np.full(NT, NT, dtype=np.int32)
    for t in range(NT):
        act = np.nonzero(state[t] != 0)[0]
        if len(act):
            s0[t] = act[0]
            for s in range(act[0], NT):
                if state[t, s] == 0:  # hole: zero it explicitly
                    state[t, s] = 2
                    tile_idx[t, s] = len(keep_tiles)
                    keep_tiles.append(np.zeros((128, 128), dtype=np.float16))
    if not keep_tiles:
        keep_tiles.append(np.zeros((128, 128), dtype=np.float16))
    return state, s0, tile_idx, np.stack(keep_tiles)


def _build_program(state, s0, tile_idx, n_keep):
    nc = bacc.Bacc(None, num_devices=NCORES)

    # ---- external I/O (16-bit operands for all PE inputs) ----
    XT = nc.dram_tensor("xt", [128, 8 * S], dt.float16, kind="ExternalInput")
    WQK = nc.dram_tensor("wqk", [128, 4 * 8 * 128], dt.float16,
                         kind="ExternalInput")
    WV = nc.dram_tensor("wv", [128, 8 * 128], dt.float16, kind="ExternalInput")
    BLOB = nc.dram_tensor("blob", [128, 21], dt.float32, kind="ExternalInput")
    ROW32 = nc.dram_tensor("row32", [1, 192], dt.float32, kind="ExternalInput")
    KEEP = nc.dram_tensor("keep", [128, n_keep * 128], dt.float16,
                          kind="ExternalInput")
    EYE = nc.dram_tensor("eye", [128, 128], dt.float32, kind="ExternalInput")
    WO = nc.dram_tensor("wo", [128, 8 * E], dt.float16, kind="ExternalInput")
    B16 = nc.dram_tensor("b16", [1, 128 + E], dt.float16, kind="ExternalInput")
    OUT = nc.dram_tensor("out_slice", [SLOC, E], dt.float32,
                         kind="ExternalOutput")

    # internal DRAM: per-head AllToAll; 256 data cols + (inv_sigma, -mu)
    a2a_in = [nc.dram_tensor(f"a2a_in{h}", [NCORES * DH, SLOC + 2], dt.float16)
              for h in range(HLOC)]
    a2a_out = [nc.dram_tensor(f"a2a_out{h}", [NCORES * DH, SLOC + 2], dt.float16)
               for h in range(HLOC)]
    groups = [list(range(NCORES))]

    Exp = mybir.ActivationFunctionType.Exp
    Ln = mybir.ActivationFunctionType.Ln
    ADD = mybir.AluOpType.add
    SUB = mybir.AluOpType.subtract
    MUL = mybir.AluOpType.mult
    BYP = mybir.AluOpType.bypass

    with tile.TileContext(nc) as tc:
        with tc.tile_pool(name="consts", bufs=1) as consts, \
             tc.tile_pool(name="qk", bufs=1) as qkp, \
             tc.tile_pool(name="vaug", bufs=1) as vaugp, \
             tc.tile_pool(name="stats", bufs=1) as statp:

            # ---- constants ----
            blob = consts.tile([128, 21], dt.float32, tag="blob")
            bqk = blob[:, 0:4]
            bv = blob[:, 4:5]
            gnw_t = blob[:, 5:13]
            gnb_t = blob[:, 13:21]
            row32 = consts.tile([1, 192], dt.float32, tag="row32")
            ones64 = row32[:, 0:64]
            b16 = consts.tile([1, 128 + E], dt.float16, tag="b16")
            ones1 = b16[:, 0:128]
            bo = b16[:, 128:128 + E]
            ones_col = consts.tile([128, 1], dt.float32, tag="ones_col")
            epsc = consts.tile([1, 1], dt.float32, tag="epsc")
            eye = consts.tile([128, 128], dt.float32, tag="eye")
            keep16 = consts.tile([128, n_keep, 128], dt.float16, tag="keep16")
            wo = consts.tile([128, 8, E], dt.float16, tag="wo")

            # persistent activation tiles
            qk_sb = [qkp.tile([128, S], dt.float16, tag=f"qk{j}", name=f"qk{j}")
                     for j in range(4)]
            v_aug = [vaugp.tile([128, NT, 65], dt.float16, tag=f"va{h}",
                                name=f"va{h}") for h in range(HLOC)]
            xn_h = [statp.tile([64, S], dt.float16, tag=f"xnh{h}",
                               name=f"xnh{h}") for h in range(HLOC)]
            ssum = statp.tile([64, HLOC, NCH], dt.float32, tag="ssum")
            ssq = statp.tile([64, HLOC, NCH], dt.float32, tag="ssq")
            vT = statp.tile([128, S], dt.float32, tag="vT")

            # ================= phase 1: projections =================
            with tc.tile_pool(name="proj", bufs=1) as projp, \
                 tc.tile_pool(name="proj_ps", bufs=2, space="PSUM") as proj_ps:
                wqk = projp.tile([128, 4, 8, 128], dt.float16, tag="wqk")
                xt = projp.tile([128, 8, S], dt.float16, tag="xt")
                wv = projp.tile([128, 8, 128], dt.float16, tag="wv")
                # DMA order: unblock the first matmuls ASAP
                nc.sync.dma_start(out=wqk[:, 0, :, :], in_=WQK[:, 0:1024]
                                  .rearrange("p (e q) -> p e q", e=8))
                nc.sync.dma_start(out=xt[:, 0, :], in_=XT[:, 0:S])
                nc.sync.dma_start(out=wqk[:, 1, :, :], in_=WQK[:, 1024:2048]
                                  .rearrange("p (e q) -> p e q", e=8))
                for e in range(1, 8):
                    nc.sync.dma_start(out=xt[:, e, :],
                                      in_=XT[:, S * e:S * (e + 1)])
                nc.sync.dma_start(out=wv[:], in_=WV[:])
                nc.sync.dma_start(out=wqk[:, 2:4, :, :],
                                  in_=WQK[:, 2048:].rearrange(
                                      "p (j e q) -> p j e q", j=2, e=8))
                nc.sync.dma_start(out=blob[:], in_=BLOB[:])
                nc.sync.dma_start(out=row32[:], in_=ROW32[:])
                nc.sync.dma_start(out=b16[:], in_=B16[:])
                nc.sync.dma_start(out=keep16[:], in_=KEEP[:])
                nc.sync.dma_start(out=eye[:], in_=EYE[:])
                nc.vector.memset(ones_col[:], 1.0)
                nc.vector.memset(epsc[:], EPS)

                # j order: q-h0, k-h0, v, q-h1, k-h1 (h0 attention unblocks
                # first). e-outer accumulate: one weight load per (j, e).
                seq = [(0, 0), (1, 1), (2, None), (3, 2), (4, 3)]
                for _, j in seq:
                    ps = proj_ps.tile([128, 4, 512], dt.float32, tag="pps")
                    for e in range(8):
                        w_e = wv[:, e, :] if j is None else wqk[:, j, e, :]
                        for sc in range(4):
                            nc.tensor.matmul(
                                ps[:, sc, :], w_e,
                                xt[:, e, 512 * sc:512 * (sc + 1)],
                                start=(e == 0), stop=(e == 7))
                    for sc in range(4):
                        if j is None:
                            nc.vector.tensor_scalar(
                                vT[:, 512 * sc:512 * (sc + 1)],
                                ps[:, sc, :], bv[:], None, ADD)
                        else:
                            nc.vector.tensor_scalar(
                                qk_sb[j][:, 512 * sc:512 * (sc + 1)],
                                ps[:, sc, :], bqk[:, j:j + 1], None, ADD)

            # load wo during the attention phase (off the critical DMA path)
            nc.sync.dma_start(out=wo[:], in_=WO[:])

            # ============ phase 2+3: v transpose + attention ============
            with tc.tile_pool(name="att_sb", bufs=1) as asb, \
                 tc.tile_pool(name="pssp", bufs=2, space="PSUM") as pssp, \
                 tc.tile_pool(name="otp", bufs=1, space="PSUM") as otp, \
                 tc.tile_pool(name="auxp", bufs=1, space="PSUM") as auxp:

                # v transposes into pss-pool slots: [d, s] -> [s, d]
                for h in range(HLOC):
                    for t in range(NT):
                        nc.vector.memset(v_aug[h][:, t, 64:65], 1.0)
                        vtp = pssp.tile([128, 64], dt.float32, tag="pss",
                                        name="vtp")
                        nc.tensor.transpose(
                            vtp[:],
                            vT[64 * h:64 * (h + 1), 128 * t:128 * (t + 1)],
                            eye[64 * h:64 * (h + 1), 64 * h:64 * (h + 1)])
                        nc.any.tensor_copy(v_aug[h][:, t, 0:64], vtp[:])

                for h in range(HLOC):
                    qT = qk_sb[2 * h]      # [0:64]=sub-1 dims, [64:128]=sub-2
                    kT = qk_sb[2 * h + 1]
                    for c in range(NCH):
                        lo, hi = 512 * c, 512 * (c + 1)
                        ctiles = range(4 * c, 4 * c + 4)
                        ts = []
                        for tt in range(NT):
                            if s0[tt] >= NT or 128 * int(s0[tt]) >= hi:
                                continue
                            if not any(state[tt, s] != 0 for s in ctiles):
                                continue
                            ts.append(tt)
                        ts.sort(key=lambda tt: int(s0[tt]))
                        if not ts:
                            nc.vector.memset(xn_h[h][:, lo:hi], 0.0)
                            nc.vector.memset(ssum[:, h, c:c + 1], 0.0)
                            nc.vector.memset(ssq[:, h, c:c + 1], 0.0)
                            continue
                        oT = otp.tile([65, 2, 512], dt.float32, tag="oT",
                                      name="oT")
                        for i, tt in enumerate(ts):
                            base = max(lo, 128 * int(s0[tt]))
                            w = hi - base
                            off = base - lo
                            pss = pssp.tile([128, 2, 512], dt.float32,
                                            tag="pss", name="pss")
                            for m in range(2):
                                p0, p1 = 64 * m, 64 * (m + 1)
                                nc.tensor.matmul(
                                    pss[:, m, 0:w],
                                    kT[p0:p1, 128 * tt:128 * (tt + 1)],
                                    qT[p0:p1, base:hi],
                                    start=True, stop=True)
                            et = asb.tile([128, 2, 512], dt.float16, tag="et",
                                          name="et", bufs=3)
                            nc.scalar.activation(et[:, :, 0:w], pss[:, :, 0:w],
                                                 Exp, scale=0.125)
                            for s in ctiles:
                                if 128 * s < base:
                                    continue
                                ki = int(tile_idx[tt, s])
                                if ki < 0:
                                    continue
                                doff = 128 * s - base
                                for m in range(2):
                                    blk = et[:, m, doff:doff + 128]
                                    nc.vector.tensor_tensor(
                                        blk, blk, keep16[:, ki, :], MUL)
                            for m in range(2):
                                nc.tensor.matmul(
                                    oT[:, m, off:off + w],
                                    v_aug[h][:, tt, :],
                                    et[:, m, 0:w],
                                    start=(i == 0), stop=(i == len(ts) - 1),
                                    skip_group_check=True)
                        # ---- normalize + combine the two matrices ----
                        rinv = asb.tile([1, 2, 512], dt.float32, tag="rinv",
                                        name="rinv", bufs=2)
                        nc.vector.reciprocal(rinv[:], oT[64:65, :, :])
                        o_sb = asb.tile([64, 2, 512], dt.float32, tag="osb",
                                        name="osb", bufs=2)
                        nc.vector.tensor_copy(o_sb[:], oT[0:64, :, :])
                        bps = auxp.tile([64, 2, 512], dt.float32, tag="aux",
                                        name="bps")
                        for m in range(2):
                            stat = ones64 if m == 0 else \
                                row32[:, 64 * (h + 1):64 * (h + 2)]
                            nc.tensor.matmul(bps[:, m, :], stat,
                                             rinv[:, m, :],
                                             start=True, stop=True)
                        t1 = asb.tile([64, 512], dt.float32, tag="t1",
                                      name="t1", bufs=2)
                        nc.vector.tensor_tensor(t1[:], o_sb[:, 0, :],
                                                bps[:, 0, :], MUL)
                        t2 = asb.tile([64, 512], dt.float32, tag="t2",
                                      name="t2", bufs=2)
                        nc.vector.tensor_tensor(t2[:], o_sb[:, 1, :],
                                                bps[:, 1, :], MUL)
                        nc.vector.scalar_tensor_tensor(
                            xn_h[h][:, lo:hi], t1[:], 1.0, t2[:], BYP, ADD,
                            accum_out=ssum[:, h, c:c + 1])
                        sqs = asb.tile([64, 512], dt.float16, tag="sqs",
                                       name="sqs")
                        nc.vector.scalar_tensor_tensor(
                            sqs[:], xn_h[h][:, lo:hi], 1.0, xn_h[h][:, lo:hi],
                            BYP, MUL, accum_out=ssq[:, h, c:c + 1])
                        # eager a2a staging: two dest blocks per chunk
                        for jj in (2 * c, 2 * c + 1):
                            nc.sync.dma_start(
                                out=a2a_in[h][64 * jj:64 * (jj + 1), 0:SLOC],
                                in_=xn_h[h][:, SLOC * jj:SLOC * (jj + 1)])

                    # ---- head end: stats -> (inv_sigma, -mu) -> a2a ----
                    stat2 = asb.tile([64, 2], dt.float32, tag="stat2",
                                     name="stat2")
                    nc.vector.tensor_reduce(stat2[:, 0:1], ssum[:, h, :],
                                            mybir.AxisListType.X, ADD)
                    nc.vector.tensor_reduce(stat2[:, 1:2], ssq[:, h, :],
                                            mybir.AxisListType.X, ADD)
                    red = auxp.tile([1, 2], dt.float32, tag="aux", name="red")
                    nc.tensor.matmul(red[:], ones_col[0:64, :], stat2[:],
                                     start=True, stop=True)
                    scal = asb.tile([1, 7], dt.float32, tag="scal", name="scal")
                    n_inv = 1.0 / (S * DH)
                    # cols: mean, E[x^2], mean^2, var, ln(var+eps), inv, negmu
                    nc.vector.tensor_scalar(scal[:, 0:2], red[:], n_inv, None,
                                            MUL)
                    nc.vector.tensor_tensor(scal[:, 2:3], scal[:, 0:1],
                                            scal[:, 0:1], MUL)
                    nc.vector.tensor_tensor(scal[:, 3:4], scal[:, 1:2],
                                            scal[:, 2:3], SUB)
                    nc.scalar.activation(scal[:, 4:5], scal[:, 3:4], Ln,
                                         bias=epsc[0:1, 0:1])
                    nc.scalar.activation(scal[:, 5:6], scal[:, 4:5], Exp,
                                         scale=-0.5)
                    nc.vector.tensor_scalar(scal[:, 6:7], scal[:, 0:1], -1.0,
                                            None, MUL)
                    bc = auxp.tile([64, 16], dt.float32, tag="aux", name="bc")
                    nc.tensor.matmul(
                        bc[:], ones64,
                        scal[:, 5:7].unsqueeze(1).broadcast_to((1, 8, 2)),
                        start=True, stop=True)
                    bc16 = asb.tile([64, 16], dt.float16, tag="bc16",
                                    name="bc16")
                    nc.any.tensor_copy(bc16[:], bc[:])
                    nc.sync.dma_start(
                        out=a2a_in[h][:, SLOC:SLOC + 2].rearrange(
                            "(j p) c -> p j c", p=64),
                        in_=bc16[:].rearrange("p (j c) -> p j c", j=8))
                    nc.gpsimd.collective_compute(
                        "AllToAll", mybir.AluOpType.bypass,
                        replica_groups=groups,
                        ins=[a2a_in[h][:]], outs=[a2a_out[h][:]])

            # ============ phase 4: affine + output Linear ==========
            with tc.tile_pool(name="fin", bufs=1) as finp, \
                 tc.tile_pool(name="f_ps", bufs=4, space="PSUM") as f_ps:
                xa = finp.tile([128, NCORES, SLOC], dt.float16, tag="xa")
                stats16 = finp.tile([128, 8, 2], dt.float16, tag="stats16")
                for h in range(HLOC):
                    nc.sync.dma_start(
                        out=xa[64 * h:64 * (h + 1), :, :],
                        in_=a2a_out[h][:, 0:SLOC].rearrange(
                            "(j p) s -> p j s", p=64))
                    nc.sync.dma_start(
                        out=stats16[64 * h:64 * (h + 1), :, :],
                        in_=a2a_out[h][:, SLOC:SLOC + 2].rearrange(
                            "(j p) c -> p j c", p=64))
                stats32 = finp.tile([128, 8, 2], dt.float32, tag="stats32")
                nc.vector.tensor_copy(stats32[:], stats16[:])
                a_t = finp.tile([128, 8, 1], dt.float32, tag="a_t")
                nc.vector.tensor_tensor(a_t[:], gnw_t.unsqueeze(2),
                                        stats32[:, :, 0:1], MUL)
                b_t = finp.tile([128, 8, 1], dt.float32, tag="b_t")
                nc.vector.tensor_tensor(b_t[:], a_t[:], stats32[:, :, 1:2],
                                        MUL)
                nc.vector.tensor_tensor(b_t[:], b_t[:], gnb_t.unsqueeze(2),
                                        ADD)
                xa2 = finp.tile([128, NCORES, SLOC], dt.float16, tag="xa2")
                for j in range(NCORES):
                    nc.vector.tensor_scalar(xa2[:, j, :], xa[:, j, :],
                                            a_t[:, j, :], b_t[:, j, :],
                                            MUL, ADD)
                out_sb = finp.tile([128, 2, E], dt.float32, tag="out_sb")
                for sh in range(2):   # two 128-row halves of the 256-row slice
                    for ec in range(2):  # two 512-col chunks of E
                        ps = f_ps.tile([128, 512], dt.float32, tag="f_ps")
                        for j in range(NCORES):
                            nc.tensor.matmul(
                                ps[:], xa2[:, j, 128 * sh:128 * (sh + 1)],
                                wo[:, j, 512 * ec:512 * (ec + 1)],
                                start=(j == 0), stop=False)
                        nc.tensor.matmul(ps[:], ones1,
                                         bo[:, 512 * ec:512 * (ec + 1)],
                                         start=False, stop=True)
                        nc.any.tensor_copy(
                            out_sb[:, sh, 512 * ec:512 * (ec + 1)], ps[:])
                for sh in range(2):
                    nc.sync.dma_start(out=OUT[128 * sh:128 * (sh + 1), :],
                                      in_=out_sb[:, sh, :])
    nc.finalize()
    return nc


def _prep_inputs(x, mask, Wq1, bq1, Wq2, bq2, Wk1, bk1, Wk2, bk2, Wv, bv,
                 lam, gn_w, gn_b, Wo, bo):
    f32 = np.float32
    f16 = np.float16
    x = np.asarray(x, f32).reshape(S, E)
    mask = np.asarray(mask, bool)
    state, s0, tile_idx, keep = _classify_mask(mask)
    key = (state.tobytes(), s0.tobytes())
    if key not in _PROGRAM_CACHE:
        _PROGRAM_CACHE[key] = _build_program(state, s0, tile_idx, len(keep))
    nc = _PROGRAM_CACHE[key]

    # [E, S] -> partition-major [128, 8*S]
    xT = np.ascontiguousarray(
        x.T.reshape(8, 128, S).transpose(1, 0, 2).reshape(128, 8 * S)).astype(f16)
    woT = np.ascontiguousarray(
        np.asarray(Wo, f32).T.reshape(8, 128, E).transpose(1, 0, 2)
        .reshape(128, 8 * E)).astype(f16)
    eye = np.eye(128, dtype=f32)
    b16 = np.concatenate([np.ones((1, 128), f32),
                          np.asarray(bo, f32).reshape(1, E)], axis=1).astype(f16)
    keep_p = np.ascontiguousarray(
        keep.transpose(1, 0, 2).reshape(128, -1))

    Wq1, Wq2, Wk1, Wk2, Wv = (np.asarray(a, f32) for a in (Wq1, Wq2, Wk1, Wk2, Wv))
    bq1, bq2, bk1, bk2, bv = (np.asarray(a, f32) for a in (bq1, bq2, bk1, bk2, bv))
    lam = np.asarray(lam, f32)
    gn_w = np.asarray(gn_w, f32)
    gn_b = np.asarray(gn_b, f32)
    gnw_t = np.ascontiguousarray(gn_w.reshape(8, 128).T)  # [128, 8]
    gnb_t = np.ascontiguousarray(gn_b.reshape(8, 128).T)

    in_maps = []
    for c in range(NCORES):
        h0, h1 = 2 * c, 2 * c + 1
        wqk = np.stack([
            np.concatenate([Wq1[h0].T, Wq2[h0].T], axis=1),
            np.concatenate([Wk1[h0].T, Wk2[h0].T], axis=1),
            np.concatenate([Wq1[h1].T, Wq2[h1].T], axis=1),
            np.concatenate([Wk1[h1].T, Wk2[h1].T], axis=1)])  # [4, 1024, 128]
        wqk_p = np.ascontiguousarray(
            wqk.reshape(4, 8, 128, 128).transpose(2, 0, 1, 3)
            .reshape(128, -1)).astype(f16)
        wv = np.concatenate([Wv[h0].T, Wv[h1].T], axis=1)  # [1024, 128]
        wv_p = np.ascontiguousarray(
            wv.reshape(8, 128, 128).transpose(1, 0, 2).reshape(128, -1)).astype(f16)
        blob = np.zeros((128, 21), f32)
        blob[:, 0] = np.concatenate([bq1[h0], bq2[h0]])
        blob[:, 1] = np.concatenate([bk1[h0], bk2[h0]])
        blob[:, 2] = np.concatenate([bq1[h1], bq2[h1]])
        blob[:, 3] = np.concatenate([bk1[h1], bk2[h1]])
        blob[:, 4] = np.concatenate([bv[h0], bv[h1]])
        blob[:, 5:13] = gnw_t
        blob[:, 13:21] = gnb_t
        row32 = np.concatenate([np.ones(64, f32),
                                np.full(64, -lam[h0], f32),
                                np.full(64, -lam[h1], f32)]).reshape(1, 192)
        in_maps.append({
            "xt": xT, "wqk": wqk_p, "wv": wv_p, "blob": blob, "row32": row32,
            "keep": keep_p, "eye": eye, "wo": woT, "b16": b16,
        })
    return nc, in_maps


def kernel(**inputs):
    import time
    nc, in_maps = _prep_inputs(**inputs)
    last = None
    for attempt in range(3):
        try:
            res = run_bass_kernel_spmd(nc, in_maps, list(range(NCORES)))
            break
        except Exception as e:  # transient device hiccups: retry
            last = e
            time.sleep(10 * (attempt + 1))
    else:
        raise last
    out = np.concatenate([res.results[c]["out_slice"] for c in range(NCORES)],
                         axis=0)
    return out.reshape(1, S, E).astype(np.float32)
